# revision 7
# baseline (speedup 1.0000x reference)
"""BranchedLinear (block-diagonal grouped GEMM) Trainium2 kernel.

Reference computation:
    x:[N, 64*32] -> reshape [N, 64, 32];  out[n,b,:] = x[n,b,:] @ W[b] + bias[b]
    -> reshape [N, 64*32]

Strategy (8 NeuronCores, data-parallel on batch):
  * Shard batch N=16384 across 8 cores (2048 rows each).
  * Host-side prep (numpy, cheap):
      - x shard is pre-transposed into feature-major tiles
        xt[g, c, p, n'] = x[512c + n', 128g + p]  (g=128-feature group, c=chunk)
        so every DMA is fully contiguous and the contraction dim (features)
        lands on SBUF partitions without any on-chip transpose.
      - W [64,32,32] is packed into a block-diagonal [128, 2048] matrix:
        each 128-col group g holds branches 4g..4g+3 as 32x32 diagonal blocks.
        A single K=128 matmul then computes 4 branches at once.
      - bias is packed output-feature-major [128, 16].
  * On-chip per core: per (group g, chunk c) ONE fp32r matmul with the
    block-diag W_g as the stationary operand and the 512-column x-transpose
    chunk as the moving operand (fp32r streams 1 row/cycle at N>=256 vs
    fp32's two half-speed passes). Output is produced transposed
    [128 f_out, 512 n]; DVE fuses the bias add with the PSUM->SBUF copy,
    and the host un-transposes the [16,4,128,512] result blocks (numpy).
  * Everything on-chip hides under the ~33 MiB/core DMA roofline.
"""

import numpy as np

# Problem shape (hardcoded per contract)
BATCH = 16384
NUM_BRANCHES = 64
IN_FEATURES = 32
OUT_FEATURES = 32
D = NUM_BRANCHES * IN_FEATURES  # 2048

NUM_CORES = 8
SHARD = BATCH // NUM_CORES  # 2048 rows per core
P = 128
GROUPS = D // P  # 16 feature groups (4 branches each)
BRANCH_PER_GROUP = P // IN_FEATURES  # 4

# per-core tiling
CHUNKS = 4  # batch chunks per core
CHUNK_N = SHARD // CHUNKS  # 512 (matmul moving free dim)

_NC_CACHE = {}


def _build_bass(chunks=CHUNKS, chunk_n=CHUNK_N, use_f32r=True):
    import concourse.mybir as mybir
    from concourse import bacc
    from concourse.tile import TileContext

    f32 = mybir.dt.float32
    fmm = mybir.dt.float32r if use_f32r else f32
    shard = chunks * chunk_n

    nc = bacc.Bacc("TRN2", target_bir_lowering=False, debug=False)
    xt = nc.dram_tensor("xt", [GROUPS, chunks, P, chunk_n], fmm, kind="ExternalInput")
    wbd = nc.dram_tensor("wbd", [P, D], fmm, kind="ExternalInput")
    biasp = nc.dram_tensor("biasp", [P, GROUPS], f32, kind="ExternalInput")
    outp = nc.dram_tensor("outp", [GROUPS, chunks, P, chunk_n], f32, kind="ExternalOutput")

    with TileContext(nc) as tc:
        with (
            tc.tile_pool(name="wpool", bufs=1) as wpool,
            tc.tile_pool(name="xpool", bufs=6) as xpool,
            tc.tile_pool(name="opool", bufs=6) as opool,
            tc.tile_pool(name="pspool", bufs=8, space="PSUM") as pspool,
        ):
            w_sb = wpool.tile([P, D], fmm, tag="w")
            nc.sync.dma_start(out=w_sb[:], in_=wbd[:])
            b_sb = wpool.tile([P, GROUPS], f32, tag="b")
            nc.sync.dma_start(out=b_sb[:], in_=biasp[:])

            for g in range(GROUPS):
                for c in range(chunks):
                    xt_t = xpool.tile([P, chunk_n], fmm, tag="xt")
                    nc.sync.dma_start(out=xt_t[:], in_=xt[:][g, c])
                    ps = pspool.tile([P, chunk_n], f32, tag="ps")
                    # out.T[f_out, n] = W_g.T-contracted block-diag matmul;
                    # stationary = W_g, moving = xT chunk (N=512)
                    nc.tensor.matmul(
                        ps[:],
                        w_sb[:, g * P : (g + 1) * P],
                        xt_t[:],
                        start=True,
                        stop=True,
                    )
                    o_t = opool.tile([P, chunk_n], f32, tag="o")
                    # fused bias add (broadcast along n) + PSUM->SBUF copyback
                    nc.vector.tensor_tensor(
                        o_t[:],
                        ps[:],
                        b_sb[:, g : g + 1].to_broadcast((P, chunk_n)),
                        mybir.AluOpType.add,
                    )
                    nc.sync.dma_start(out=outp[:][g, c], in_=o_t[:])
    nc.compile()
    return nc


def _get_nc(chunks=CHUNKS, chunk_n=CHUNK_N, use_f32r=True):
    key = (chunks, chunk_n, use_f32r)
    if key not in _NC_CACHE:
        _NC_CACHE[key] = _build_bass(chunks, chunk_n, use_f32r)
    return _NC_CACHE[key]


def _pack_wbd(W):
    """[64, 32, 32] -> block-diagonal [128, 2048]."""
    W = np.asarray(W, np.float32)
    wbd = np.zeros((P, D), np.float32)
    for g in range(GROUPS):
        for j in range(BRANCH_PER_GROUP):
            b = g * BRANCH_PER_GROUP + j
            r0 = j * IN_FEATURES
            c0 = g * P + j * OUT_FEATURES
            wbd[r0 : r0 + IN_FEATURES, c0 : c0 + OUT_FEATURES] = W[b]
    return wbd


def _pack_xt(shard, chunks=CHUNKS, chunk_n=CHUNK_N):
    """[shard_n, 2048] -> [GROUPS, chunks, 128, chunk_n] feature-major tiles."""
    return np.ascontiguousarray(
        shard.reshape(chunks, chunk_n, GROUPS, P).transpose(2, 0, 3, 1)
    )


def _pack_bias(b):
    """[64, 32] -> [128, GROUPS] output-feature-major."""
    return np.ascontiguousarray(
        np.asarray(b, np.float32).reshape(GROUPS, P).T
    )


def _unpack_out(outp, chunks=CHUNKS, chunk_n=CHUNK_N):
    """[GROUPS, chunks, 128, chunk_n] -> [shard_n, 2048]."""
    return outp.transpose(1, 3, 0, 2).reshape(chunks * chunk_n, D)


def kernel(x, W, b):
    from concourse.bass_utils import run_bass_kernel_spmd

    x = np.asarray(x, np.float32)
    wbd = _pack_wbd(W)
    biasp = _pack_bias(b)

    nc = _get_nc()
    in_maps = []
    for i in range(NUM_CORES):
        shard = x[i * SHARD : (i + 1) * SHARD]
        in_maps.append({"xt": _pack_xt(shard), "wbd": wbd, "biasp": biasp})

    res = run_bass_kernel_spmd(nc, in_maps, core_ids=list(range(NUM_CORES)))
    return np.concatenate(
        [_unpack_out(r["outp"]) for r in res.results], axis=0
    )


# revision 10
# speedup vs baseline: 1.3208x; 1.3208x over previous
"""BranchedLinear (block-diagonal grouped GEMM) Trainium2 kernel.

Reference computation:
    x:[N, 64*32] -> reshape [N, 64, 32];  out[n,b,:] = x[n,b,:] @ W[b] + bias[b]
    -> reshape [N, 64*32]

Strategy (8 NeuronCores, data-parallel on batch):
  * Shard batch N=16384 across 8 cores (2048 rows each).
  * Host-side prep (numpy, cheap):
      - x shard is pre-transposed into feature-major tiles
        xt[g, c, p, n'] = x[512c + n', 128g + p]  (g=128-feature group, c=chunk)
        so every DMA is fully contiguous and the contraction dim (features)
        lands on SBUF partitions without any on-chip transpose.
      - W [64,32,32] is packed into a block-diagonal [128, 2048] matrix:
        each 128-col group g holds branches 4g..4g+3 as 32x32 diagonal blocks.
        A single K=128 matmul then computes 4 branches at once.
      - bias is packed output-feature-major [128, 16].
  * On-chip per core: per (group g, chunk c) ONE fp32r matmul with the
    block-diag W_g as the stationary operand and the 512-column x-transpose
    chunk as the moving operand (fp32r streams 1 row/cycle at N>=256 vs
    fp32's two half-speed passes). Output is produced transposed
    [128 f_out, 512 n]; DVE fuses the bias add with the PSUM->SBUF copy,
    and the host un-transposes the [16,4,128,512] result blocks (numpy).
  * Everything on-chip hides under the ~33 MiB/core DMA roofline.
"""

import numpy as np

# Problem shape (hardcoded per contract)
BATCH = 16384
NUM_BRANCHES = 64
IN_FEATURES = 32
OUT_FEATURES = 32
D = NUM_BRANCHES * IN_FEATURES  # 2048

NUM_CORES = 8
SHARD = BATCH // NUM_CORES  # 2048 rows per core
P = 128
GROUPS = D // P  # 16 feature groups (4 branches each)
BRANCH_PER_GROUP = P // IN_FEATURES  # 4

# per-core tiling
CHUNKS = 4  # batch chunks per core
CHUNK_N = SHARD // CHUNKS  # 512 (matmul moving free dim)

_NC_CACHE = {}


def _build_bass(chunks=CHUNKS, chunk_n=CHUNK_N, use_f32r=True):
    import concourse.mybir as mybir
    from concourse import bacc
    from concourse.tile import TileContext

    f32 = mybir.dt.float32
    fmm = mybir.dt.float32r if use_f32r else f32
    shard = chunks * chunk_n

    nc = bacc.Bacc("TRN2", target_bir_lowering=False, debug=False)
    xt = nc.dram_tensor("xt", [GROUPS, P, shard], fmm, kind="ExternalInput")
    wbd = nc.dram_tensor("wbd", [P, D], fmm, kind="ExternalInput")
    biasp = nc.dram_tensor("biasp", [P, GROUPS], f32, kind="ExternalInput")
    outp = nc.dram_tensor("outp", [GROUPS, P, shard], f32, kind="ExternalOutput")

    with TileContext(nc) as tc:
        with (
            tc.tile_pool(name="wpool", bufs=1) as wpool,
            tc.tile_pool(name="xpool", bufs=4) as xpool,
            tc.tile_pool(name="opool", bufs=4) as opool,
            tc.tile_pool(name="pspool", bufs=2, space="PSUM") as pspool,
        ):
            b_sb = wpool.tile([P, GROUPS], f32, tag="b")
            nc.sync.dma_start(out=b_sb[:], in_=biasp[:])
            w_sb = wpool.tile([P, D], fmm, tag="w")
            nc.sync.dma_start(out=w_sb[:], in_=wbd[:])

            for g in range(GROUPS):
                # whole group strip [128 f, shard n]: 8 KB/partition DMA
                xt_t = xpool.tile([P, shard], fmm, tag="xt")
                nc.sync.dma_start(out=xt_t[:], in_=xt[:][g])
                # 4-bank PSUM tile; each chunk's matmul fills one bank
                ps = pspool.tile([P, shard], f32, tag="ps")
                for c in range(chunks):
                    # out.T[f_out, n] block; stationary = block-diag W_g,
                    # moving = xT chunk (N=512, fp32r streams 1 row/cycle)
                    nc.tensor.matmul(
                        ps[:, c * chunk_n : (c + 1) * chunk_n],
                        w_sb[:, g * P : (g + 1) * P],
                        xt_t[:, c * chunk_n : (c + 1) * chunk_n],
                        start=True,
                        stop=True,
                    )
                o_t = opool.tile([P, shard], f32, tag="o")
                # fused bias add (broadcast along n) + PSUM->SBUF copyback
                nc.vector.tensor_tensor(
                    o_t[:],
                    ps[:],
                    b_sb[:, g : g + 1].to_broadcast((P, shard)),
                    mybir.AluOpType.add,
                )
                # stores ride the ACT HWDGE ring, loads the SP ring
                nc.scalar.dma_start(out=outp[:][g], in_=o_t[:])
    nc.compile()
    return nc


def _get_nc(chunks=CHUNKS, chunk_n=CHUNK_N, use_f32r=True):
    key = (chunks, chunk_n, use_f32r)
    if key not in _NC_CACHE:
        _NC_CACHE[key] = _build_bass(chunks, chunk_n, use_f32r)
    return _NC_CACHE[key]


def _pack_wbd(W):
    """[64, 32, 32] -> block-diagonal [128, 2048]."""
    W = np.asarray(W, np.float32)
    wbd = np.zeros((P, D), np.float32)
    for g in range(GROUPS):
        for j in range(BRANCH_PER_GROUP):
            b = g * BRANCH_PER_GROUP + j
            r0 = j * IN_FEATURES
            c0 = g * P + j * OUT_FEATURES
            wbd[r0 : r0 + IN_FEATURES, c0 : c0 + OUT_FEATURES] = W[b]
    return wbd


def _pack_xt(shard, chunks=CHUNKS, chunk_n=CHUNK_N):
    """[shard_n, 2048] -> [GROUPS, 128, shard_n] feature-major strips."""
    n = shard.shape[0]
    return np.ascontiguousarray(shard.T).reshape(GROUPS, P, n)


def _pack_bias(b):
    """[64, 32] -> [128, GROUPS] output-feature-major."""
    return np.ascontiguousarray(
        np.asarray(b, np.float32).reshape(GROUPS, P).T
    )


def _unpack_out(outp, chunks=CHUNKS, chunk_n=CHUNK_N):
    """[GROUPS, 128, shard_n] -> [shard_n, 2048]."""
    return outp.reshape(D, chunks * chunk_n).T


def kernel(x, W, b):
    from concourse.bass_utils import run_bass_kernel_spmd

    x = np.asarray(x, np.float32)
    wbd = _pack_wbd(W)
    biasp = _pack_bias(b)

    nc = _get_nc()
    in_maps = []
    for i in range(NUM_CORES):
        shard = x[i * SHARD : (i + 1) * SHARD]
        in_maps.append({"xt": _pack_xt(shard), "wbd": wbd, "biasp": biasp})

    res = run_bass_kernel_spmd(nc, in_maps, core_ids=list(range(NUM_CORES)))
    return np.concatenate(
        [_unpack_out(r["outp"]) for r in res.results], axis=0
    )
